# revision 1
# baseline (speedup 1.0000x reference)
"""Trainium2 Bass kernel for nn_Attention_10015863734775.

Multi-head causal attention (16 heads, d_model 2048, d_head 128, seq 2048,
batch 1) with llama-style interleaved RoPE and a signed-softmax:
    attn_w = sign(s) * exp(|s| - max|s|);  attn = attn_w / (sum|attn_w| + 1e-6)
The max-subtraction cancels in the normalization (scores are O(5), exp is
safe in fp32), so the device computes attn = sign(s)exp(|s|) / sum exp(|s|).

Sharding: 2 heads per NeuronCore (8 cores). Each core receives the full
transposed residual X^T plus its head slices of W_Q/K/V/W_O and computes a
partial output projection outT_c[m, s]; the host sums the 8 partials,
transposes, and adds b_O (exact: b_O enters after all nonlinearities).

Device layouts are all [feature, seq] ("T" layouts) so that:
  - scoresT[k, q] blocks come straight from matmul(lhsT=kT chunk, rhs=qT)
  - the z matmul needs no transposes (V is transposed once via the PE)
  - the signed-softmax k-sum is a ones-vector matmul on the PE
Causal masking skips fully-masked blocks; diagonal blocks add a -1e5 mask
to |s| before exp (exp underflows to exactly 0).
All matmuls run as float32r (~tf32 operand rounding, full fp32 accumulate).
"""

import math

import numpy as np

S = 2048          # sequence length
D = 2048          # d_model
DH = 128          # d_head
NH = 16           # total heads
NC = 8            # neuron cores
HPC = NH // NC    # heads per core (2)
ST = 512          # seq tile (matmul free dim / one PSUM bank)
NST = S // ST     # 4 seq tiles
NDC = D // 128    # 16 contraction chunks
NKC = S // 128    # 16 k chunks
C_SCALE = 1.0 / math.sqrt(float(DH))
LN2 = math.log(2.0)
MASK_NEG = -1.0e5

_CACHE = {}


def _build_program():
    import concourse.tile as tile
    from concourse import bacc, mybir

    F32 = mybir.dt.float32
    F32R = mybir.dt.float32r
    AF = mybir.ActivationFunctionType
    ALU = mybir.AluOpType

    nc = bacc.Bacc("TRN2", target_bir_lowering=False, debug=False, num_devices=NC)

    xt_d = nc.dram_tensor("xt", [D, S], F32, kind="ExternalInput").ap()
    wq_d = nc.dram_tensor("wq", [HPC, D, DH], F32, kind="ExternalInput").ap()
    wk_d = nc.dram_tensor("wk", [HPC, D, DH], F32, kind="ExternalInput").ap()
    wv_d = nc.dram_tensor("wv", [HPC, D, DH], F32, kind="ExternalInput").ap()
    wo_d = nc.dram_tensor("wo", [HPC, DH, D], F32, kind="ExternalInput").ap()
    bq_d = nc.dram_tensor("bq", [HPC, DH, 1], F32, kind="ExternalInput").ap()
    bk_d = nc.dram_tensor("bk", [HPC, DH, 1], F32, kind="ExternalInput").ap()
    bv_d = nc.dram_tensor("bv", [HPC, DH, 1], F32, kind="ExternalInput").ap()
    cos_d = nc.dram_tensor("cost", [DH, S], F32, kind="ExternalInput").ap()
    sin_d = nc.dram_tensor("sint", [DH, S], F32, kind="ExternalInput").ap()
    msk_d = nc.dram_tensor("maskneg", [128, 896], F32, kind="ExternalInput").ap()
    pt_d = nc.dram_tensor("pt", [128, 128], F32, kind="ExternalInput").ap()
    id_d = nc.dram_tensor("ident", [128, 128], F32, kind="ExternalInput").ap()
    oc_d = nc.dram_tensor("onescol", [128, 1], F32, kind="ExternalInput").ap()
    or_d = nc.dram_tensor("onesrow", [1, 128], F32, kind="ExternalInput").ap()
    out_d = nc.dram_tensor("outt", [D, S], F32, kind="ExternalOutput").ap()

    with tile.TileContext(nc) as tc:
        with tc.tile_pool(name="persist", bufs=1) as pp:
            # persistent SBUF
            wo_sb = []
            bqs, bks, bvs = [], [], []
            qrot, krot, v_sb, znt = [], [], [], []
            for h in range(HPC):
                t = pp.tile([DH, D], F32R, tag=f"wo{h}")
                nc.sync.dma_start(t[:], wo_d[h].bitcast(F32R))
                wo_sb.append(t)
                for lst, dd, nm in ((bqs, bq_d, "bq"), (bks, bk_d, "bk"), (bvs, bv_d, "bv")):
                    bt = pp.tile([DH, 1], F32, tag=f"{nm}{h}")
                    nc.sync.dma_start(bt[:], dd[h])
                    lst.append(bt)
                qrot.append(pp.tile([DH, S], F32R, tag=f"qrot{h}"))
                krot.append(pp.tile([DH, S], F32R, tag=f"krot{h}"))
                v_sb.append(pp.tile([128, NKC, DH], F32R, tag=f"v{h}"))
                znt.append(pp.tile([DH, S], F32R, tag=f"znt{h}"))
            msk_sb = pp.tile([128, 896], F32, tag="msk")
            nc.sync.dma_start(msk_sb[:], msk_d[:])
            pt_sb = pp.tile([128, 128], F32R, tag="pt")
            nc.sync.dma_start(pt_sb[:], pt_d[:].bitcast(F32R))
            id_sb = pp.tile([128, 128], F32R, tag="ident")
            nc.sync.dma_start(id_sb[:], id_d[:].bitcast(F32R))
            oc_sb = pp.tile([128, 1], F32R, tag="onescol")
            nc.sync.dma_start(oc_sb[:], oc_d[:].bitcast(F32R))
            or_sb = pp.tile([1, 128], F32R, tag="onesrow")
            nc.sync.dma_start(or_sb[:], or_d[:].bitcast(F32R))

            # ---------------- Phase A: projections + RoPE + V transpose ----
            with tc.tile_pool(name="aphase", bufs=1) as ap_, \
                 tc.tile_pool(name="axt", bufs=3) as axt, \
                 tc.tile_pool(name="aev", bufs=2) as aev, \
                 tc.tile_pool(name="arope", bufs=3) as arp, \
                 tc.tile_pool(name="psA", bufs=1, space="PSUM") as psA, \
                 tc.tile_pool(name="psShuf", bufs=1, space="PSUM") as psSh, \
                 tc.tile_pool(name="psVtr", bufs=1, space="PSUM") as psVt:
                w_sb = {}
                for key, dd in (("q", wq_d), ("k", wk_d), ("v", wv_d)):
                    for h in range(HPC):
                        t = ap_.tile([128, NDC, DH], F32R, tag=f"w{key}{h}")
                        nc.sync.dma_start(
                            t[:], dd[h].rearrange("(c p) e -> p c e", p=128).bitcast(F32R)
                        )
                        w_sb[(key, h)] = t
                cos_sb = ap_.tile([DH, S], F32, tag="cos")
                nc.sync.dma_start(cos_sb[:], cos_d[:])
                sin_sb = ap_.tile([DH, S], F32, tag="sin")
                nc.sync.dma_start(sin_sb[:], sin_d[:])

                for st in range(NST):
                    ssl = slice(st * ST, (st + 1) * ST)
                    acc = {}
                    for key in ("q", "k", "v"):
                        for h in range(HPC):
                            acc[(key, h)] = psA.tile([128, ST], F32, tag=f"acc{key}{h}")
                    for dc in range(NDC):
                        xt_t = axt.tile([128, ST], F32R, tag="xt")
                        nc.sync.dma_start(
                            xt_t[:],
                            xt_d[dc * 128:(dc + 1) * 128, ssl].bitcast(F32R),
                        )
                        for key in ("q", "k", "v"):
                            for h in range(HPC):
                                nc.tensor.matmul(
                                    acc[(key, h)][:], w_sb[(key, h)][:, dc, :], xt_t[:],
                                    start=(dc == 0), stop=(dc == NDC - 1),
                                )
                    for h in range(HPC):
                        # q / k: bias-evict then RoPE
                        for key, bias, dst in (("q", bqs[h], qrot[h]), ("k", bks[h], krot[h])):
                            x_sb = aev.tile([128, ST], F32R, tag="ev")
                            nc.scalar.activation(x_sb[:], acc[(key, h)][:], AF.Identity, bias=bias[:])
                            shuf = psSh.tile([128, ST], F32, tag="shuf")
                            nc.tensor.matmul(shuf[:].bitcast(F32R), pt_sb[:], x_sb[:],
                                             start=True, stop=True)
                            t1 = arp.tile([128, ST], F32, tag="t1")
                            nc.gpsimd.tensor_tensor(t1[:], x_sb[:].bitcast(F32), cos_sb[:, ssl], ALU.mult)
                            t2 = arp.tile([128, ST], F32, tag="t2")
                            nc.vector.tensor_tensor(t2[:], shuf[:], sin_sb[:, ssl], ALU.mult)
                            nc.vector.tensor_tensor(dst[:, ssl], t1[:], t2[:], ALU.add)
                        # v: bias-evict then transpose to [s, d] chunks
                        vt_sb = aev.tile([128, ST], F32R, tag="evv")
                        nc.scalar.activation(vt_sb[:], acc[("v", h)][:], AF.Identity, bias=bvs[h][:])
                        for sc in range(ST // 128):
                            vtr = psVt.tile([128, 128], F32, tag="vtr")
                            nc.tensor.transpose(vtr[:].bitcast(F32R),
                                                vt_sb[:, sc * 128:(sc + 1) * 128], id_sb[:])
                            nc.vector.tensor_copy(v_sb[h][:, st * 4 + sc, :], vtr[:])

            # ---------------- Phases B + C --------------------------------
            with tc.tile_pool(name="bwork", bufs=2) as bw, \
                 tc.tile_pool(name="bsmall", bufs=2) as bsm, \
                 tc.tile_pool(name="cout", bufs=3) as co, \
                 tc.tile_pool(name="psS", bufs=2, space="PSUM") as psS, \
                 tc.tile_pool(name="psZ", bufs=2, space="PSUM") as psZ, \
                 tc.tile_pool(name="psD", bufs=2, space="PSUM") as psD, \
                 tc.tile_pool(name="psRB", bufs=1, space="PSUM") as psRB, \
                 tc.tile_pool(name="psO", bufs=1, space="PSUM") as psO:
                for j in range(NST):
                    jsl = slice(j * ST, (j + 1) * ST)
                    for h in range(HPC):
                        nkc = 4 * (j + 1)
                        psz = psZ.tile([128, ST], F32, tag="z")
                        psd = psD.tile([1, ST], F32, tag="d")
                        for kc in range(nkc):
                            pss = psS.tile([128, ST], F32, tag="s")
                            nc.tensor.matmul(pss[:], krot[h][:, kc * 128:(kc + 1) * 128],
                                             qrot[h][:, jsl], start=True, stop=True)
                            a = bw.tile([128, ST], F32, tag="a")
                            jj = kc - 4 * j
                            if 0 <= jj < 4:
                                nc.vector.scalar_tensor_tensor(
                                    a[:], pss[:], 0.0,
                                    msk_sb[:, 384 - 128 * jj: 896 - 128 * jj],
                                    ALU.abs_max, ALU.add)
                            else:
                                nc.vector.tensor_scalar(a[:], pss[:], 0.0, None, ALU.abs_max)
                            e2 = bw.tile([128, ST], F32R, tag="e2")
                            nc.scalar.activation(e2[:], a[:], AF.Exp, bias=LN2, scale=C_SCALE)
                            w = bw.tile([128, ST], F32R, tag="w")
                            if kc % 2 == 0:
                                sg = bw.tile([128, ST], F32, tag="sg")
                                nc.scalar.activation(sg[:], pss[:], AF.Sign)
                                nc.gpsimd.tensor_tensor(w[:], sg[:], e2[:].bitcast(F32), ALU.mult)
                            else:
                                g2 = bw.tile([128, ST], F32, tag="sg")
                                nc.vector.tensor_scalar(g2[:], pss[:], 0.0, 2.0, ALU.is_ge, ALU.mult)
                                nc.gpsimd.scalar_tensor_tensor(w[:], g2[:], 1.0, e2[:].bitcast(F32),
                                                               ALU.subtract, ALU.mult)
                            nc.tensor.matmul(psd[:], oc_sb[:], e2[:],
                                             start=(kc == 0), stop=(kc == nkc - 1))
                            nc.tensor.matmul(psz[:], v_sb[h][:, kc, :], w[:],
                                             start=(kc == 0), stop=(kc == nkc - 1))
                        d_sb = bsm.tile([1, ST], F32, tag="dsb")
                        nc.vector.tensor_copy(d_sb[:], psd[:])
                        r_sb = bsm.tile([1, ST], F32R, tag="rsb")
                        with nc.allow_low_precision(reason="f32r recip for broadcast"):
                            nc.vector.reciprocal(r_sb[:], d_sb[:])
                        psrb = psRB.tile([128, ST], F32, tag="rb")
                        nc.tensor.matmul(psrb[:].bitcast(F32R), or_sb[:], r_sb[:],
                                         start=True, stop=True)
                        rb_sb = bw.tile([128, ST], F32, tag="rb")
                        nc.scalar.activation(rb_sb[:], psrb[:], AF.Copy)
                        nc.vector.tensor_tensor(znt[h][:, jsl], psz[:], rb_sb[:], ALU.mult)
                    # phase C for this j
                    for mc in range(D // 128):
                        pso = psO.tile([128, ST], F32, tag="o")
                        for h in range(HPC):
                            nc.tensor.matmul(pso[:], wo_sb[h][:, mc * 128:(mc + 1) * 128],
                                             znt[h][:, jsl], start=(h == 0), stop=(h == HPC - 1))
                        o_sb = co.tile([128, ST], F32, tag="o")
                        if mc % 2 == 0:
                            nc.vector.tensor_copy(o_sb[:], pso[:])
                        else:
                            nc.scalar.activation(o_sb[:], pso[:], AF.Copy)
                        nc.sync.dma_start(out_d[mc * 128:(mc + 1) * 128, jsl], o_sb[:])

    nc.compile()
    return nc


def _host_constants():
    inv = 1.0 / (10000.0 ** (np.arange(0, DH, 2, dtype=np.float32) / DH))
    t = np.arange(S, dtype=np.float32)
    fr = t[:, None] * inv[None, :]                       # [S, DH/2]
    cosT = np.repeat(np.cos(fr).astype(np.float32).T, 2, axis=0)  # [DH, S]
    sinT = np.repeat(np.sin(fr).astype(np.float32).T, 2, axis=0)

    # sliding causal mask: msk[k, c] = 0 if k <= c - 384 else MASK_NEG
    kk = np.arange(128)[:, None]
    cc = np.arange(896)[None, :]
    msk = np.where(kk <= cc - 384, 0.0, MASK_NEG).astype(np.float32)

    # pt = P.T with P@x the rotate-half shuffle: (P x)[2i] = -x[2i+1], (P x)[2i+1] = x[2i]
    pt = np.zeros((128, 128), dtype=np.float32)
    i = np.arange(0, 128, 2)
    pt[i + 1, i] = -1.0
    pt[i, i + 1] = 1.0

    ident = np.eye(128, dtype=np.float32)
    onescol = np.ones((128, 1), dtype=np.float32)
    onesrow = np.ones((1, 128), dtype=np.float32)
    return cosT, sinT, msk, pt, ident, onescol, onesrow


def _run(inputs, trace=False, trace_kwargs=None):
    from concourse.bass_utils import run_bass_kernel_spmd

    if "nc" not in _CACHE:
        _CACHE["nc"] = _build_program()
    nc = _CACHE["nc"]

    resid_pre = np.asarray(inputs["resid_pre"], dtype=np.float32)
    W_Q = np.asarray(inputs["W_Q"], dtype=np.float32)
    W_K = np.asarray(inputs["W_K"], dtype=np.float32)
    W_V = np.asarray(inputs["W_V"], dtype=np.float32)
    W_O = np.asarray(inputs["W_O"], dtype=np.float32)
    b_Q = np.asarray(inputs["b_Q"], dtype=np.float32)
    b_K = np.asarray(inputs["b_K"], dtype=np.float32)
    b_V = np.asarray(inputs["b_V"], dtype=np.float32)
    b_O = np.asarray(inputs["b_O"], dtype=np.float32)

    xt = np.ascontiguousarray(resid_pre[0].T)
    cosT, sinT, msk, pt, ident, onescol, onesrow = _host_constants()

    in_maps = []
    for c in range(NC):
        hs = slice(c * HPC, (c + 1) * HPC)
        in_maps.append({
            "xt": xt,
            "wq": np.ascontiguousarray(W_Q[hs]),
            "wk": np.ascontiguousarray(W_K[hs]),
            "wv": np.ascontiguousarray(W_V[hs]),
            "wo": np.ascontiguousarray(W_O[hs]),
            "bq": np.ascontiguousarray(b_Q[hs][:, :, None]),
            "bk": np.ascontiguousarray(b_K[hs][:, :, None]),
            "bv": np.ascontiguousarray(b_V[hs][:, :, None]),
            "cost": cosT, "sint": sinT, "maskneg": msk, "pt": pt,
            "ident": ident, "onescol": onescol, "onesrow": onesrow,
        })

    kw = dict(trace_kwargs or {})
    res = run_bass_kernel_spmd(nc, in_maps, list(range(NC)), trace=trace, **kw)

    acc = np.zeros((D, S), dtype=np.float32)
    for c in range(NC):
        acc += res.results[c]["outt"]
    out = acc.T + b_O[None, :]
    return out.reshape(1, S, D).astype(np.float32), res


def kernel(**inputs) -> np.ndarray:
    out, _ = _run(inputs, trace=False)
    return out


# revision 2
# speedup vs baseline: 1.0613x; 1.0613x over previous
"""Trainium2 Bass kernel for nn_Attention_10015863734775.

Multi-head causal attention (16 heads, d_model 2048, d_head 128, seq 2048,
batch 1) with llama-style interleaved RoPE and a signed-softmax:
    attn_w = sign(s) * exp(|s| - max|s|);  attn = attn_w / (sum|attn_w| + 1e-6)
The max-subtraction cancels in the normalization (scores are O(5), exp is
safe in fp32), so the device computes attn = sign(s)exp(|s|) / sum exp(|s|).

Sharding: 2 heads per NeuronCore (8 cores). Each core receives the full
transposed residual X^T plus its head slices of W_Q/K/V/W_O and computes a
partial output projection outT_c[m, s]; the host sums the 8 partials,
transposes, and adds b_O (exact: b_O enters after all nonlinearities).

Device layouts are all [feature, seq] ("T" layouts) so that:
  - scoresT[k, q] blocks come straight from matmul(lhsT=kT chunk, rhs=qT)
  - the z matmul needs no transposes (V is transposed once via the PE)
  - the signed-softmax k-sum is a ones-vector matmul on the PE
Causal masking skips fully-masked blocks; diagonal blocks add a -1e5 mask
to |s| before exp (exp underflows to exactly 0).
All matmuls run as float32r (~tf32 operand rounding, full fp32 accumulate).
"""

import math

import numpy as np

S = 2048          # sequence length
D = 2048          # d_model
DH = 128          # d_head
NH = 16           # total heads
NC = 8            # neuron cores
HPC = NH // NC    # heads per core (2)
ST = 512          # seq tile (matmul free dim / one PSUM bank)
NST = S // ST     # 4 seq tiles
NDC = D // 128    # 16 contraction chunks
NKC = S // 128    # 16 k chunks
C_SCALE = 1.0 / math.sqrt(float(DH))
LN2 = math.log(2.0)
MASK_NEG = -1.0e5

_CACHE = {}


def _build_program():
    import concourse.tile as tile
    from concourse import bacc, mybir

    F32 = mybir.dt.float32
    F32R = mybir.dt.float32r
    AF = mybir.ActivationFunctionType
    ALU = mybir.AluOpType

    nc = bacc.Bacc("TRN2", target_bir_lowering=False, debug=False, num_devices=NC)

    xt_d = nc.dram_tensor("xt", [D, S], F32, kind="ExternalInput").ap()
    wq_d = nc.dram_tensor("wq", [HPC, D, DH], F32, kind="ExternalInput").ap()
    wk_d = nc.dram_tensor("wk", [HPC, D, DH], F32, kind="ExternalInput").ap()
    wv_d = nc.dram_tensor("wv", [HPC, D, DH], F32, kind="ExternalInput").ap()
    wo_d = nc.dram_tensor("wo", [HPC, DH, D], F32, kind="ExternalInput").ap()
    bq_d = nc.dram_tensor("bq", [HPC, DH, 1], F32, kind="ExternalInput").ap()
    bk_d = nc.dram_tensor("bk", [HPC, DH, 1], F32, kind="ExternalInput").ap()
    bv_d = nc.dram_tensor("bv", [HPC, DH, 1], F32, kind="ExternalInput").ap()
    cos_d = nc.dram_tensor("cost", [DH, S], F32, kind="ExternalInput").ap()
    sin_d = nc.dram_tensor("sint", [DH, S], F32, kind="ExternalInput").ap()
    msk_d = nc.dram_tensor("maskneg", [128, 896], F32, kind="ExternalInput").ap()
    pt_d = nc.dram_tensor("pt", [128, 128], F32, kind="ExternalInput").ap()
    id_d = nc.dram_tensor("ident", [128, 128], F32, kind="ExternalInput").ap()
    oc_d = nc.dram_tensor("onescol", [128, 1], F32, kind="ExternalInput").ap()
    or_d = nc.dram_tensor("onesrow", [1, 128], F32, kind="ExternalInput").ap()
    out_d = nc.dram_tensor("outt", [D, S], F32, kind="ExternalOutput").ap()

    with tile.TileContext(nc) as tc:
        with tc.tile_pool(name="persist", bufs=1) as pp:
            # persistent SBUF
            wo_sb = []
            bqs, bks, bvs = [], [], []
            qrot, krot, v_sb, znt = [], [], [], []
            for h in range(HPC):
                t = pp.tile([DH, D], F32R, tag=f"wo{h}")
                nc.sync.dma_start(t[:], wo_d[h].bitcast(F32R))
                wo_sb.append(t)
                for lst, dd, nm in ((bqs, bq_d, "bq"), (bks, bk_d, "bk"), (bvs, bv_d, "bv")):
                    bt = pp.tile([DH, 1], F32, tag=f"{nm}{h}")
                    nc.sync.dma_start(bt[:], dd[h])
                    lst.append(bt)
                qrot.append(pp.tile([DH, S], F32R, tag=f"qrot{h}"))
                krot.append(pp.tile([DH, S], F32R, tag=f"krot{h}"))
                v_sb.append(pp.tile([128, NKC, DH], F32R, tag=f"v{h}"))
                znt.append(pp.tile([DH, S], F32R, tag=f"znt{h}"))
            msk_sb = pp.tile([128, 896], F32, tag="msk")
            nc.sync.dma_start(msk_sb[:], msk_d[:])
            pt_sb = pp.tile([128, 128], F32R, tag="pt")
            nc.sync.dma_start(pt_sb[:], pt_d[:].bitcast(F32R))
            id_sb = pp.tile([128, 128], F32R, tag="ident")
            nc.sync.dma_start(id_sb[:], id_d[:].bitcast(F32R))
            oc_sb = pp.tile([128, 1], F32R, tag="onescol")
            nc.sync.dma_start(oc_sb[:], oc_d[:].bitcast(F32R))
            or_sb = pp.tile([1, 128], F32R, tag="onesrow")
            nc.sync.dma_start(or_sb[:], or_d[:].bitcast(F32R))

            # ---------------- Phase A: projections + RoPE + V transpose ----
            with tc.tile_pool(name="aphase", bufs=1) as ap_, \
                 tc.tile_pool(name="axt", bufs=3) as axt, \
                 tc.tile_pool(name="aev", bufs=2) as aev, \
                 tc.tile_pool(name="arope", bufs=3) as arp, \
                 tc.tile_pool(name="psA", bufs=1, space="PSUM") as psA, \
                 tc.tile_pool(name="psShuf", bufs=1, space="PSUM") as psSh, \
                 tc.tile_pool(name="psVtr", bufs=1, space="PSUM") as psVt:
                w_sb = {}
                for key, dd in (("q", wq_d), ("k", wk_d), ("v", wv_d)):
                    for h in range(HPC):
                        t = ap_.tile([128, NDC, DH], F32R, tag=f"w{key}{h}")
                        nc.sync.dma_start(
                            t[:], dd[h].rearrange("(c p) e -> p c e", p=128).bitcast(F32R)
                        )
                        w_sb[(key, h)] = t
                cos_sb = ap_.tile([DH, S], F32, tag="cos")
                nc.sync.dma_start(cos_sb[:], cos_d[:])
                sin_sb = ap_.tile([DH, S], F32, tag="sin")
                nc.sync.dma_start(sin_sb[:], sin_d[:])

                for st in range(NST):
                    ssl = slice(st * ST, (st + 1) * ST)
                    acc = {}
                    for key in ("q", "k", "v"):
                        for h in range(HPC):
                            acc[(key, h)] = psA.tile([128, ST], F32, tag=f"acc{key}{h}")
                    for dc in range(NDC):
                        xt_t = axt.tile([128, ST], F32R, tag="xt")
                        nc.sync.dma_start(
                            xt_t[:],
                            xt_d[dc * 128:(dc + 1) * 128, ssl].bitcast(F32R),
                        )
                        for key in ("q", "k", "v"):
                            for h in range(HPC):
                                nc.tensor.matmul(
                                    acc[(key, h)][:], w_sb[(key, h)][:, dc, :], xt_t[:],
                                    start=(dc == 0), stop=(dc == NDC - 1),
                                )
                    for h in range(HPC):
                        # q / k: bias-evict then RoPE
                        for key, bias, dst in (("q", bqs[h], qrot[h]), ("k", bks[h], krot[h])):
                            x_sb = aev.tile([128, ST], F32R, tag="ev")
                            nc.scalar.activation(x_sb[:], acc[(key, h)][:], AF.Identity, bias=bias[:])
                            shuf = psSh.tile([128, ST], F32, tag="shuf")
                            nc.tensor.matmul(shuf[:].bitcast(F32R), pt_sb[:], x_sb[:],
                                             start=True, stop=True)
                            t1 = arp.tile([128, ST], F32, tag="t1")
                            nc.gpsimd.tensor_tensor(t1[:], x_sb[:].bitcast(F32), cos_sb[:, ssl], ALU.mult)
                            t2 = arp.tile([128, ST], F32, tag="t2")
                            nc.vector.tensor_tensor(t2[:], shuf[:], sin_sb[:, ssl], ALU.mult)
                            nc.vector.tensor_tensor(dst[:, ssl], t1[:], t2[:], ALU.add)
                        # v: bias-evict then transpose to [s, d] chunks
                        vt_sb = aev.tile([128, ST], F32R, tag="evv")
                        nc.scalar.activation(vt_sb[:], acc[("v", h)][:], AF.Identity, bias=bvs[h][:])
                        for sc in range(ST // 128):
                            vtr = psVt.tile([128, 128], F32, tag="vtr")
                            nc.tensor.transpose(vtr[:].bitcast(F32R),
                                                vt_sb[:, sc * 128:(sc + 1) * 128], id_sb[:])
                            nc.vector.tensor_copy(v_sb[h][:, st * 4 + sc, :], vtr[:])

            # ---------------- Phases B + C --------------------------------
            with tc.tile_pool(name="bwork", bufs=2) as bw, \
                 tc.tile_pool(name="bsmall", bufs=2) as bsm, \
                 tc.tile_pool(name="cout", bufs=3) as co, \
                 tc.tile_pool(name="psS", bufs=2, space="PSUM") as psS, \
                 tc.tile_pool(name="psZ", bufs=2, space="PSUM") as psZ, \
                 tc.tile_pool(name="psD", bufs=2, space="PSUM") as psD, \
                 tc.tile_pool(name="psRB", bufs=1, space="PSUM") as psRB, \
                 tc.tile_pool(name="psO", bufs=1, space="PSUM") as psO:
                for j in range(NST):
                    jsl = slice(j * ST, (j + 1) * ST)
                    for h in range(HPC):
                        nkc = 4 * (j + 1)
                        psz = psZ.tile([128, ST], F32, tag="z")
                        psd = psD.tile([1, ST], F32, tag="d")
                        for kc in range(nkc):
                            pss = psS.tile([128, ST], F32, tag="s")
                            nc.tensor.matmul(pss[:], krot[h][:, kc * 128:(kc + 1) * 128],
                                             qrot[h][:, jsl], start=True, stop=True)
                            a = bw.tile([128, ST], F32, tag="a")
                            jj = kc - 4 * j
                            if 0 <= jj < 4:
                                nc.vector.scalar_tensor_tensor(
                                    a[:], pss[:], 0.0,
                                    msk_sb[:, 384 - 128 * jj: 896 - 128 * jj],
                                    ALU.abs_max, ALU.add)
                            else:
                                nc.vector.tensor_scalar(a[:], pss[:], 0.0, None, ALU.abs_max)
                            e2 = bw.tile([128, ST], F32R, tag="e2")
                            nc.scalar.activation(e2[:], a[:], AF.Exp, bias=LN2, scale=C_SCALE)
                            w = bw.tile([128, ST], F32R, tag="w")
                            if kc % 2 == 0:
                                sg = bw.tile([128, ST], F32, tag="sg")
                                nc.scalar.activation(sg[:], pss[:], AF.Sign)
                                nc.gpsimd.tensor_tensor(w[:], sg[:], e2[:].bitcast(F32), ALU.mult)
                            else:
                                g2 = bw.tile([128, ST], F32, tag="sg")
                                nc.vector.tensor_scalar(g2[:], pss[:], 0.0, 2.0, ALU.is_ge, ALU.mult)
                                nc.gpsimd.scalar_tensor_tensor(w[:], g2[:], 1.0, e2[:].bitcast(F32),
                                                               ALU.subtract, ALU.mult)
                            nc.tensor.matmul(psd[:], oc_sb[:], e2[:],
                                             start=(kc == 0), stop=(kc == nkc - 1))
                            nc.tensor.matmul(psz[:], v_sb[h][:, kc, :], w[:],
                                             start=(kc == 0), stop=(kc == nkc - 1))
                        d_sb = bsm.tile([1, ST], F32, tag="dsb")
                        nc.vector.tensor_copy(d_sb[:], psd[:])
                        r_sb = bsm.tile([1, ST], F32R, tag="rsb")
                        with nc.allow_low_precision(reason="f32r recip for broadcast"):
                            nc.vector.reciprocal(r_sb[:], d_sb[:])
                        psrb = psRB.tile([128, ST], F32, tag="rb")
                        nc.tensor.matmul(psrb[:].bitcast(F32R), or_sb[:], r_sb[:],
                                         start=True, stop=True)
                        rb_sb = bw.tile([128, ST], F32, tag="rb")
                        nc.scalar.activation(rb_sb[:], psrb[:], AF.Copy)
                        nc.vector.tensor_tensor(znt[h][:, jsl], psz[:], rb_sb[:], ALU.mult)
                    # phase C for this j
                    for mc in range(D // 128):
                        pso = psO.tile([128, ST], F32, tag="o")
                        for h in range(HPC):
                            nc.tensor.matmul(pso[:], wo_sb[h][:, mc * 128:(mc + 1) * 128],
                                             znt[h][:, jsl], start=(h == 0), stop=(h == HPC - 1))
                        o_sb = co.tile([128, ST], F32, tag="o")
                        if mc % 2 == 0:
                            nc.vector.tensor_copy(o_sb[:], pso[:])
                        else:
                            nc.scalar.activation(o_sb[:], pso[:], AF.Copy)
                        nc.sync.dma_start(out_d[mc * 128:(mc + 1) * 128, jsl], o_sb[:])

    nc.compile()
    return nc


def _host_constants():
    inv = 1.0 / (10000.0 ** (np.arange(0, DH, 2, dtype=np.float32) / DH))
    t = np.arange(S, dtype=np.float32)
    fr = t[:, None] * inv[None, :]                       # [S, DH/2]
    cosT = np.repeat(np.cos(fr).astype(np.float32).T, 2, axis=0)  # [DH, S]
    sinT = np.repeat(np.sin(fr).astype(np.float32).T, 2, axis=0)

    # sliding causal mask: msk[k, c] = 0 if k <= c - 384 else MASK_NEG
    kk = np.arange(128)[:, None]
    cc = np.arange(896)[None, :]
    msk = np.where(kk <= cc - 384, 0.0, MASK_NEG).astype(np.float32)

    # pt = P.T with P@x the rotate-half shuffle: (P x)[2i] = -x[2i+1], (P x)[2i+1] = x[2i]
    pt = np.zeros((128, 128), dtype=np.float32)
    i = np.arange(0, 128, 2)
    pt[i + 1, i] = -1.0
    pt[i, i + 1] = 1.0

    ident = np.eye(128, dtype=np.float32)
    onescol = np.ones((128, 1), dtype=np.float32)
    onesrow = np.ones((1, 128), dtype=np.float32)
    return cosT, sinT, msk, pt, ident, onescol, onesrow


def _run(inputs, trace=False, trace_kwargs=None):
    from concourse.bass_utils import run_bass_kernel_spmd

    if "nc" not in _CACHE:
        _CACHE["nc"] = _build_program()
    nc = _CACHE["nc"]

    resid_pre = np.asarray(inputs["resid_pre"], dtype=np.float32)
    W_Q = np.asarray(inputs["W_Q"], dtype=np.float32)
    W_K = np.asarray(inputs["W_K"], dtype=np.float32)
    W_V = np.asarray(inputs["W_V"], dtype=np.float32)
    W_O = np.asarray(inputs["W_O"], dtype=np.float32)
    b_Q = np.asarray(inputs["b_Q"], dtype=np.float32)
    b_K = np.asarray(inputs["b_K"], dtype=np.float32)
    b_V = np.asarray(inputs["b_V"], dtype=np.float32)
    b_O = np.asarray(inputs["b_O"], dtype=np.float32)

    xt = np.ascontiguousarray(resid_pre[0].T)
    cosT, sinT, msk, pt, ident, onescol, onesrow = _host_constants()

    in_maps = []
    for c in range(NC):
        hs = slice(c * HPC, (c + 1) * HPC)
        in_maps.append({
            "xt": xt,
            "wq": np.ascontiguousarray(W_Q[hs]),
            "wk": np.ascontiguousarray(W_K[hs]),
            "wv": np.ascontiguousarray(W_V[hs]),
            "wo": np.ascontiguousarray(W_O[hs]),
            "bq": np.ascontiguousarray(b_Q[hs][:, :, None]),
            "bk": np.ascontiguousarray(b_K[hs][:, :, None]),
            "bv": np.ascontiguousarray(b_V[hs][:, :, None]),
            "cost": cosT, "sint": sinT, "maskneg": msk, "pt": pt,
            "ident": ident, "onescol": onescol, "onesrow": onesrow,
        })

    kw = dict(trace_kwargs or {})
    last_err = None
    for attempt in range(3):
        try:
            res = run_bass_kernel_spmd(nc, in_maps, list(range(NC)), trace=trace, **kw)
            break
        except Exception as e:  # transient NRT_EXEC_UNIT_UNRECOVERABLE wedges clear on retry
            last_err = e
            if attempt == 2 or "UNRECOVERABLE" not in str(e).upper() and "UNAVAILABLE" not in str(e).upper():
                raise
            import time
            time.sleep(3.0)
    else:
        raise last_err

    acc = np.zeros((D, S), dtype=np.float32)
    for c in range(NC):
        acc += res.results[c]["outt"]
    out = acc.T + b_O[None, :]
    return out.reshape(1, S, D).astype(np.float32), res


def kernel(**inputs) -> np.ndarray:
    out, _ = _run(inputs, trace=False)
    return out
